# revision 1
# baseline (speedup 1.0000x reference)
"""Trainium2 Bass kernel for CausalSelfAttention (QK-RMSNorm + RoPE).

Sharding: 8 cores = 2 batches x 4 head-groups (4 heads each).
Each core computes QKV projection for its heads, attention, and a partial
output projection (row-parallel c_proj); host sums the 4 partials per batch
and adds b_proj.

Structure vs the original baseline: batched DMAs spread across engine
queues (x^T and weights in bf16 to halve HBM traffic), software-pipelined
PE queue (transposes trail two t-blocks; S^T matmuls lead PV by one
key-block; output projection woven into the attention phase), per-head PV
accumulators so the softmax-normalize chain pipelines across head-pairs,
and diagonal S matmuls restricted to the causal column range.  The
attention-core dtypes (q/k^T, exp(S), V, O^T) stay f32r — 2-byte broadcast
or in-place elementwise ops misbehave on real HW engines.

Self-contained: hardcodes shapes B=2, T=2048, D=1024, H=16, HD=64.
"""
import os
import sys
import math
from contextlib import ExitStack

for _p in ("/opt/trn_rl_repo", "/root/.axon_site/_ro/trn_rl_repo"):
    if os.path.isdir(_p) and _p not in sys.path:
        sys.path.append(_p)

import numpy as np

import concourse.bass as bass
import concourse.bacc as bacc
import concourse.mybir as mybir
import concourse.tile as tile
from concourse.vector_clock import ScopedClock
from concourse.bass_utils import run_bass_kernel_spmd

B, T, D = 2, 2048, 1024
H, HD = 16, 64
EPS = 1e-6
NCORES = 8
HPC = 4          # heads per core
F = HPC * HD     # 256 features per core per q/k/v
NTB = T // 128   # 16 t-blocks
NIB = T // 512   # 4 i-blocks
F32 = mybir.dt.float32
F32R = mybir.dt.float32r
BF = mybir.dt.bfloat16
AF = mybir.ActivationFunctionType
ALU = mybir.AluOpType
AX = mybir.AxisListType


class TileContextSplitDrain(tile.TileContext):
    """Work around walrus 'Too many sync wait commands' on the tail drain:
    split the global-clock waits across single-wait SP drains."""

    MAX_DRAIN_WAITS = 1

    def _drain_and_barrier(self, tick_clock, wait_clock):
        drain_inst = self.nc.sync.drain()
        wait_clock.add_sem_waits(
            drain_inst.ins, ScopedClock({None: tick_clock.global_clock})
        )
        si = drain_inst.ins.sync_info
        waits = list(si.on_wait) if si and si.on_wait else []
        si.on_wait = waits[: self.MAX_DRAIN_WAITS]
        rest = waits[self.MAX_DRAIN_WAITS:]
        while rest:
            d = self.nc.sync.drain()
            d.ins.sync_info = mybir.SyncInfo(
                on_wait=rest[: self.MAX_DRAIN_WAITS], on_update=[]
            )
            rest = rest[self.MAX_DRAIN_WAITS:]

        self.nc.all_engine_barrier()
        assert self.sems is not None
        popped = self.nc._tile_sem_poison_stack.pop()
        assert popped is self._sem_poison
        self.nc.clear_and_free_semaphores(list(self.sems.allocated().values()))
        self.nc.all_engine_barrier()


def build_program(exp_bias: float, has_bias: bool = True):
    """One SPMD program; per-core behavior differs only via input data."""
    nc = bacc.Bacc("TRN2", target_bir_lowering=False)
    xt8 = nc.declare_dram_parameter("xt8", [128, 8 * T], BF, isOutput=False)
    wqkv8 = nc.declare_dram_parameter("wqkv8", [128, 8 * 3 * F], BF, isOutput=False)
    bqkv = nc.declare_dram_parameter("bqkv", [1, 3 * F], F32, isOutput=False)
    cgqk = nc.declare_dram_parameter("cgqk", [128, NTB * 128], BF, isOutput=False)
    sgqk = nc.declare_dram_parameter("sgqk", [128, NTB * 128], BF, isOutput=False)
    wp2 = nc.declare_dram_parameter("wp2", [128, 2 * D], F32R, isOutput=False)
    trimask = nc.declare_dram_parameter("trimask", [128, 4 * 512], BF, isOutput=False)
    ident = nc.declare_dram_parameter("ident", [128, 128], F32R, isOutput=False)
    onesd = nc.declare_dram_parameter("onesd", [128, 128], F32R, isOutput=False)
    outp = nc.declare_dram_parameter("outp", [T, D], F32, isOutput=True)

    with tile.TileContext(nc) as tc, ExitStack() as ctx:
        cpool = ctx.enter_context(tc.tile_pool(name="consts", bufs=1))
        workq = ctx.enter_context(tc.tile_pool(name="workq", bufs=2))
        qrp = ctx.enter_context(tc.tile_pool(name="qrp", bufs=3))
        smallp = ctx.enter_context(tc.tile_pool(name="smallp", bufs=2))
        epool = ctx.enter_context(tc.tile_pool(name="epool", bufs=3))
        rvp = ctx.enter_context(tc.tile_pool(name="rvp", bufs=2))
        stp = ctx.enter_context(tc.tile_pool(name="stp", bufs=2))
        # PSUM: 8 banks total.  ps1: 2 x 1-bank (pq/pv in phase A; pj and the
        # denominator-broadcast tiles in B/C); ps2: 2 x 2-bank (transposes in
        # A, S^T tiles in B); pop: 2 x 1-bank (per-head PV accumulators).
        ps1 = ctx.enter_context(tc.tile_pool(name="ps1", bufs=2, space="PSUM"))
        ps2 = ctx.enter_context(tc.tile_pool(name="ps2", bufs=2, space="PSUM"))
        pop = ctx.enter_context(tc.tile_pool(name="pop", bufs=2, space="PSUM"))

        # ---- persistent SBUF ----
        xt_sb = cpool.tile([128, 8, T], BF, tag="xt")
        wq_sb = cpool.tile([128, 8, 3 * F], BF, tag="wq")
        cg_sb = cpool.tile([128, NTB, 128], BF, tag="cg")
        sg_sb = cpool.tile([128, NTB, 128], BF, tag="sg")
        wp_sb = cpool.tile([128, 2, D], F32R, tag="wp")
        tri_sb = cpool.tile([128, 4, 512], BF, tag="tri")
        id_sb = cpool.tile([128, 128], F32R, tag="ident")
        onesP = cpool.tile([128, 128], F32R, tag="onesP")
        eps_b = cpool.tile([128, 1], F32, tag="epsb")
        ebias_b = cpool.tile([128, 1], F32, tag="ebiasb")
        qkt = cpool.tile([128, 4, T], F32R, tag="qkt")   # [c(2 heads), {q,k}, t]
        v4 = cpool.tile([128, NTB, HPC, HD + 1], F32R, tag="v4")  # V|1
        ont = cpool.tile([128, 2, T], F32R, tag="ont")   # normalized O^T chunks
        bb = cpool.tile([128, 3 * F], F32, tag="bb") if has_bias else None

        # ---- prologue DMAs (spread across engine queues) ----
        wqv = wqkv8.rearrange("p (k f) -> p k f", k=8)
        nc.scalar.dma_start(out=wq_sb[:, 0:2, :], in_=wqv[:, 0:2, :])
        nc.scalar.dma_start(out=wq_sb[:, 2:4, :], in_=wqv[:, 2:4, :])
        nc.scalar.dma_start(out=wq_sb[:, 4:6, :], in_=wqv[:, 4:6, :])
        nc.scalar.dma_start(out=wq_sb[:, 6:8, :], in_=wqv[:, 6:8, :])
        xtv = xt8.rearrange("p (k t) -> p k t", k=8)
        nc.sync.dma_start(out=xt_sb[:, :, 0:256], in_=xtv[:, :, 0:256])
        nc.sync.dma_start(out=xt_sb[:, :, 256:512], in_=xtv[:, :, 256:512])
        nc.gpsimd.dma_start(out=cg_sb, in_=cgqk.rearrange("p (t c) -> p t c", t=NTB))
        nc.gpsimd.dma_start(out=sg_sb, in_=sgqk.rearrange("p (t c) -> p t c", t=NTB))
        nc.scalar.dma_start(out=id_sb, in_=ident[:, :])
        nc.scalar.dma_start(out=onesP, in_=onesd[:, :])
        nc.vector.memset(eps_b, EPS)
        nc.vector.memset(ebias_b, float(exp_bias))
        # V ones-columns (softmax denominator trick), written once
        nc.scalar.copy(
            out=v4[:, :, :, HD:HD + 1],
            in_=onesP[:, 0:NTB * HPC].rearrange("p (t h) -> p t h", t=NTB).unsqueeze(3),
        )
        if has_bias:
            bq_in = stp.tile([1, 3 * F], F32, tag="bqin")
            nc.gpsimd.dma_start(out=bq_in, in_=bqkv[:, :])
            nc.gpsimd.partition_broadcast(bb, bq_in)
        # staged xt chunks + phase-B constants on the SP queue
        nc.sync.dma_start(out=xt_sb[:, :, 512:1024], in_=xtv[:, :, 512:1024])
        nc.sync.dma_start(out=wp_sb, in_=wp2.rearrange("p (k f) -> p k f", k=2))
        nc.sync.dma_start(out=xt_sb[:, :, 1024:1536], in_=xtv[:, :, 1024:1536])
        nc.sync.dma_start(out=tri_sb, in_=trimask.rearrange("p (r c) -> p r c", r=4))
        nc.sync.dma_start(out=xt_sb[:, :, 1536:2048], in_=xtv[:, :, 1536:2048])

        # ---- phase A: QKV projection + rmsnorm + rope; transposes trail ----
        def emit_qkv(tb):
            ts = slice(tb * 128, (tb + 1) * 128)
            pq = ps1.tile([128, 512], F32, tag="ps1", name=f"pq{tb}")
            pv = ps1.tile([128, 256], F32, tag="ps1", name=f"pv{tb}")
            for kd in range(8):
                nc.tensor.matmul(pq, xt_sb[:, kd, ts], wq_sb[:, kd, 0:512],
                                 start=(kd == 0), stop=(kd == 7))
            for kd in range(8):
                nc.tensor.matmul(pv, xt_sb[:, kd, ts], wq_sb[:, kd, 512:768],
                                 start=(kd == 0), stop=(kd == 7))
            # stage q|k to SBUF in f32 (Act; Copy is in every act table)
            pqs = workq.tile([128, 512], F32, tag="pqs", name=f"pqs{tb}")
            nc.scalar.copy(out=pqs, in_=pq)
            if has_bias:
                nc.vector.tensor_add(pqs, pqs, bb[:, 0:512])
            # V -> SBUF [t, head, c]
            pvv = pv.rearrange("p (h c) -> p h c", h=HPC)
            if has_bias:
                bbv = bb[:, 512:768].rearrange("p (h c) -> p h c", h=HPC)
                nc.vector.tensor_tensor(v4[:, tb, :, 0:HD], pvv, bbv, ALU.add)
            else:
                nc.scalar.copy(out=v4[:, tb, :, 0:HD], in_=pvv)
            # rmsnorm stats (Pool does the square; DVE is A's pacer)
            sq = workq.tile([128, 512], F32, tag="sq", name=f"sq{tb}")
            nc.gpsimd.tensor_mul(sq, pqs, pqs)
            var = smallp.tile([128, 8], F32, tag="var", name=f"var{tb}")
            nc.vector.tensor_reduce(
                var, sq.rearrange("p (h c) -> p h c", h=8), AX.X, ALU.add
            )
            rstd_s = smallp.tile([128, 8], F32, tag="rstds", name=f"rstds{tb}")
            nc.scalar.activation(rstd_s, var, AF.Sqrt, scale=1.0 / HD,
                                 bias=eps_b[:, :])
            rstd = smallp.tile([128, 8], F32, tag="rstd", name=f"rstd{tb}")
            nc.vector.reciprocal(rstd, rstd_s)
            # qn = q * rstd (per-head broadcast), f32 in -> bf16 out
            qn = workq.tile([128, 512], BF, tag="qn", name=f"qn{tb}")
            nc.vector.tensor_tensor(
                qn.rearrange("p (h c) -> p h c", h=8),
                pqs.rearrange("p (h c) -> p h c", h=8),
                rstd.unsqueeze(2).broadcast_to((128, 8, HD)),
                ALU.mult,
            )
            # rope: qr = qn*CG + shift(qn)*SG   (bf16 multiplies, f32r sum)
            qn4 = qn.rearrange("p (g h c) -> p g h c", g=2, h=HPC)
            cgs = cg_sb[:, tb, :].rearrange("p (g c) -> p g c", g=2)
            sgs = sg_sb[:, tb, :].rearrange("p (g c) -> p g c", g=2)
            m1 = workq.tile([128, 512], BF, tag="m1", name=f"m1_{tb}")
            m1v = m1.rearrange("p (g h c) -> p g h c", g=2, h=HPC)
            nc.vector.tensor_tensor(
                m1v, qn4, cgs.unsqueeze(2).broadcast_to((128, 2, HPC, HD)), ALU.mult
            )
            m2 = workq.tile([128, 512], BF, tag="m2", name=f"m2_{tb}")
            m2v = m2.rearrange("p (g h c) -> p g h c", g=2, h=HPC)
            nc.vector.tensor_tensor(
                m2v[:, :, :, 0:32],
                qn4[:, :, :, 32:64],
                sgs[:, :, 0:32].unsqueeze(2).broadcast_to((128, 2, HPC, 32)),
                ALU.mult,
            )
            nc.vector.tensor_tensor(
                m2v[:, :, :, 32:64],
                qn4[:, :, :, 0:32],
                sgs[:, :, 32:64].unsqueeze(2).broadcast_to((128, 2, HPC, 32)),
                ALU.mult,
            )
            qr = qrp.tile([128, 512], F32R, tag="qr", name=f"qr{tb}")
            nc.vector.tensor_add(qr, m1, m2)
            return qr

        def emit_tr(tb, qr):
            ts = slice(tb * 128, (tb + 1) * 128)
            tr = ps2.tile([128, 8, 128], F32R, tag="ps2", name=f"tr{tb}")
            for cc in range(4):
                nc.tensor.transpose(tr[:, cc, :], qr[:, cc * 128:(cc + 1) * 128],
                                    id_sb)
            nc.scalar.copy(out=qkt[:, :, ts], in_=tr[:, 0:4, :])

        qrs = {}
        for tb in range(NTB):
            if tb >= 2:
                emit_tr(tb - 2, qrs.pop(tb - 2))
            qrs[tb] = emit_qkv(tb)
        emit_tr(NTB - 2, qrs.pop(NTB - 2))
        emit_tr(NTB - 1, qrs.pop(NTB - 1))

        # ---- phase B: attention (jc-pipelined), with C woven in ----
        def emit_sp(hp, ib, jc):
            # moving >= 256 rows keeps f32r at 1 cyc/row; exp only reads
            # [s_true:512] so the extra columns are harmless
            s = min(max(0, 128 * (jc - 4 * ib)), 256)
            isl = slice(ib * 512 + s, (ib + 1) * 512)
            jsl = slice(jc * 128, (jc + 1) * 128)
            sp = ps2.tile([128, 2, 512], F32, tag="ps2", name=f"sp{hp}_{ib}_{jc}")
            nc.tensor.matmul(
                sp[:, 0, s:512], qkt[0:64, 2 + hp, jsl], qkt[0:64, hp, isl],
                start=True, stop=True, tile_position=(0, 0),
            )
            nc.tensor.matmul(
                sp[:, 1, s:512], qkt[64:128, 2 + hp, jsl], qkt[64:128, hp, isl],
                start=True, stop=True, tile_position=(64, 0),
            )
            return sp

        def emit_fin(hp, ib, jc, sp, po, njc):
            s = max(0, 128 * (jc - 4 * ib))
            e = epool.tile([128, 2, 512], F32R, tag="e", name=f"e{hp}_{ib}_{jc}")
            nc.scalar.activation(
                e[:, :, s:512], sp[:, :, s:512], AF.Exp,
                scale=1.0 / math.sqrt(HD), bias=ebias_b[:, :],
            )
            if jc >= 4 * ib:  # diagonal-crossing tile: triangle mask
                r = jc - 4 * ib
                nc.gpsimd.tensor_mul(
                    e[:, :, s:s + 128],
                    e[:, :, s:s + 128],
                    tri_sb[:, r, s:s + 128].unsqueeze(1).broadcast_to((128, 2, 128)),
                )
            first, last = (jc == 0), (jc == njc - 1)
            for h in range(2):
                head = hp * 2 + h
                nc.tensor.matmul(
                    po[h][:, s:512], v4[:, jc, head, :], e[:, h, s:512],
                    start=first, stop=last,
                )

        def emit_norm(hp, ib, po):
            isl = slice(ib * 512, (ib + 1) * 512)
            rv = rvp.tile([128, 1024], F32R, tag="rv", name=f"rv{hp}_{ib}")
            pos = [rvp.tile([65, 512], F32, tag="pos", name=f"pos{hp}_{ib}_{h}")
                   for h in range(2)]
            for h in range(2):
                with nc.allow_low_precision(reason="f32r softmax denom"):
                    nc.vector.reciprocal(
                        rv[64:65, h * 512:(h + 1) * 512], po[h][64:65, :]
                    )
            nc.scalar.copy(out=pos[0], in_=po[0])
            nc.vector.tensor_copy(pos[1], po[1])
            pb = [ps1.tile([64, 512], F32, tag="ps1", name=f"pb{hp}_{ib}_{h}")
                  for h in range(2)]
            for h in range(2):
                nc.tensor.matmul(
                    pb[h], onesP[64:65, 0:64], rv[64:65, h * 512:(h + 1) * 512],
                    start=True, stop=True,
                )
            nc.vector.tensor_mul(ont[0:64, hp, isl], pos[0][0:64, :], pb[0])
            stage = stp.tile([64, 512], F32R, tag="stage", name=f"st{hp}_{ib}")
            nc.vector.tensor_mul(stage, pos[1][0:64, :], pb[1])
            eng = nc.scalar if (ib == NIB - 1 and hp == 1) else nc.sync
            eng.dma_start(out=ont[64:128, hp, isl], in_=stage)

        obs = {}

        def emit_chalf(tb, nh, k, tail=False):
            ts = slice(tb * 128, (tb + 1) * 128)
            pj = ps1.tile([128, 512], F32, tag="ps1", name=f"pj{tb}_{nh}")
            for hp in range(2):
                nc.tensor.matmul(
                    pj, ont[:, hp, ts], wp_sb[:, hp, nh * 512:(nh + 1) * 512],
                    start=(hp == 0), stop=(hp == 1),
                )
            if nh == 0:
                obs[tb] = stp.tile([128, D], F32, tag="ob", name=f"ob{tb}")
            ob = obs[tb]
            if tail and nh == 1:
                nc.scalar.copy(out=ob[:, nh * 512:(nh + 1) * 512], in_=pj)
                nc.scalar.dma_start(out=outp[ts, :], in_=obs.pop(tb))
            else:
                nc.vector.tensor_copy(ob[:, nh * 512:(nh + 1) * 512], pj)
                if nh == 1:
                    nc.sync.dma_start(out=outp[ts, :], in_=obs.pop(tb))

        ck = 0
        for ib in range(NIB):
            njc = 4 * ib + 4
            halves = []
            if ib > 0:
                halves = [(tb, nh) for tb in range(4 * (ib - 1), 4 * ib)
                          for nh in range(2)]
            hidx = 0
            for hp in range(2):
                po = [pop.tile([65, 512], F32, tag="po", name=f"po{hp}_{ib}_{h}")
                      for h in range(2)]
                sp_prev = emit_sp(hp, ib, 0)
                for jc in range(1, njc):
                    sp_cur = emit_sp(hp, ib, jc)
                    emit_fin(hp, ib, jc - 1, sp_prev, po, njc)
                    sp_prev = sp_cur
                    if jc % 2 == 1 and hidx < len(halves):
                        tb, nh = halves[hidx]
                        hidx += 1
                        emit_chalf(tb, nh, ck)
                        ck += 1
                emit_fin(hp, ib, njc - 1, sp_prev, po, njc)
                emit_norm(hp, ib, po)
            while hidx < len(halves):
                tb, nh = halves[hidx]
                hidx += 1
                emit_chalf(tb, nh, ck)
                ck += 1
        for tb in range(12, 16):
            for nh in range(2):
                emit_chalf(tb, nh, ck, tail=True)
                ck += 1

    nc.compile()
    return nc


def host_inputs(x, w_attn, b_attn, w_proj, g_q, g_k, rope_cos, rope_sin):
    """Per-core input maps + exp bias."""
    import ml_dtypes
    bf16 = ml_dtypes.bfloat16
    x = np.asarray(x, dtype=np.float32)
    w_attn = np.asarray(w_attn, dtype=np.float32)
    b_attn = np.asarray(b_attn, dtype=np.float32)
    w_proj = np.asarray(w_proj, dtype=np.float32)
    g_q = np.asarray(g_q, dtype=np.float32)
    g_k = np.asarray(g_k, dtype=np.float32)
    rope_cos = np.asarray(rope_cos, dtype=np.float32)
    rope_sin = np.asarray(rope_sin, dtype=np.float32)

    # |s| <= 8 * max|g_q| * max|g_k| after RMSNorm; subtract for exp safety
    bound = 8.0 * max(1e-6, float(np.abs(g_q).max())) * max(
        1e-6, float(np.abs(g_k).max())
    )
    exp_bias = -bound

    # rope tables with gains folded in; shifted-sign sin for rotate_half
    def sg_of(g):
        sgn = np.where(np.arange(HD) < HD // 2, -1.0, 1.0).astype(np.float32)
        gperm = np.roll(g, HD // 2)  # g[(c+32)%64]
        return rope_sin * (sgn * gperm)[None, :]  # [T, HD]

    cgq = rope_cos * g_q[None, :]
    cgk = rope_cos * g_k[None, :]
    sgq = sg_of(g_q)
    sgk = sg_of(g_k)

    def arrange_rope(a_q, a_k):
        # [T, HD] x2 -> [128, NTB*128] with [p, tb, {q:64 | k:64}]
        aq = a_q.reshape(NTB, 128, HD).transpose(1, 0, 2)
        ak = a_k.reshape(NTB, 128, HD).transpose(1, 0, 2)
        return np.ascontiguousarray(
            np.concatenate([aq, ak], axis=2).reshape(128, NTB * 128)
        ).astype(bf16)

    cg_arr = arrange_rope(cgq, cgk)
    sg_arr = arrange_rope(sgq, sgk)

    # masks[j, r, :]: zeros for cols < 128r, triu(j <= i') on cols [128r,128r+128)
    tri = np.zeros((128, 4, 512), dtype=np.float32)
    for r in range(4):
        tri[:, r, 128 * r:128 * (r + 1)] = np.triu(np.ones((128, 128), np.float32))
        tri[:, r, 128 * (r + 1):] = 1.0
    tri = np.ascontiguousarray(tri.reshape(128, 4 * 512)).astype(bf16)
    ident = np.eye(128, dtype=np.float32)

    in_maps = []
    for c in range(NCORES):
        b, hg = divmod(c, 4)
        f0 = hg * F
        rows = np.concatenate([
            np.arange(f0, f0 + F),
            D + np.arange(f0, f0 + F),
            2 * D + np.arange(f0, f0 + F),
        ])
        w = w_attn[rows]                      # [768, 1024]
        wqkvT = np.ascontiguousarray(w.T)     # [1024, 768]
        wqkv8 = np.ascontiguousarray(
            wqkvT.reshape(8, 128, 3 * F).transpose(1, 0, 2).reshape(128, 8 * 3 * F)
        ).astype(bf16)
        bq = np.ascontiguousarray(b_attn[rows].reshape(1, 3 * F))
        wpT = np.ascontiguousarray(w_proj[:, f0:f0 + F].T)  # [256, 1024]
        wp2 = np.ascontiguousarray(
            wpT.reshape(2, 128, D).transpose(1, 0, 2).reshape(128, 2 * D)
        )
        xtT = np.ascontiguousarray(x[b].T)    # [1024, 2048]
        xt8 = np.ascontiguousarray(
            xtT.reshape(8, 128, T).transpose(1, 0, 2).reshape(128, 8 * T)
        ).astype(bf16)
        in_maps.append({
            "xt8": xt8,
            "wqkv8": wqkv8,
            "bqkv": bq,
            "cgqk": cg_arr,
            "sgqk": sg_arr,
            "wp2": wp2,
            "trimask": tri,
            "ident": ident,
            "onesd": np.ones((128, 128), dtype=np.float32),
        })
    return in_maps, exp_bias


_CACHE = {}


def kernel(x, w_attn, b_attn, w_proj, b_proj, g_q, g_k, rope_cos, rope_sin):
    in_maps, exp_bias = host_inputs(
        x, w_attn, b_attn, w_proj, g_q, g_k, rope_cos, rope_sin
    )
    has_bias = bool(np.any(np.asarray(b_attn)))
    key = (float(exp_bias), has_bias)
    if key not in _CACHE:
        _CACHE[key] = build_program(exp_bias, has_bias)
    nc = _CACHE[key]
    res = run_bass_kernel_spmd(nc, in_maps, list(range(NCORES)))
    out = np.zeros((B, T, D), dtype=np.float32)
    for c in range(NCORES):
        out[c // 4] += np.asarray(res.results[c]["outp"], dtype=np.float32)
    out += np.asarray(b_proj, dtype=np.float32)[None, None, :]
    return out

